# revision 1
# baseline (speedup 1.0000x reference)
"""Trainium2 Bass kernel for nn_AttentionFlow (BiDAF-style attention flow).

Math (per batch b, all biases cancel):
  s[t,i]   = <c_t,w_c> + <q_i,w_q> + <c_t*q_i, w_cq>  (+ biases)
  a        = softmax_i(s)          -> c2q = a @ q
  beta     = softmax_t(max_i s)    -> q2c = beta^T c
  out      = [c | c2q | c*c2q | c*q2c]

Key identities:
  * softmax_i(s[t,:]) is invariant to the per-row term sc[t] and all biases.
  * beta depends only on sc_raw[t] + max_i(sq_raw[i] + scq_raw[t,i]).
  * sc is folded into the matmul rhs:  qa[d,i] = q^T[d,i]*w_cq[d] + w_c[d].
  * t and i orderings are arbitrary (softmax/sums are order-invariant and
    outputs are re-addressed by AP), so row->partition maps are chosen for
    DMA contiguity when opts['contig_in'] is set.

Sharding: data-parallel over batch, one batch element per NeuronCore (8).
"""

import numpy as np

N_CORES = 8
T, I, D = 2048, 512, 512
TT = T // 128  # 16 row tiles
KC = 4         # 128-chunks of D (and of I)

DEFAULT_OPTS = dict(contig_in=True, out_ring="sync", two_pass=False,
                    skip_out=False, memset_in=False, dma_pair=False,
                    bufs_work=3, bufs_out=3, out_split=False,
                    dup_pe=False, dup_dve=False, dup_act=False, bloat=0,
                    act_copies="act", exp_accum=False, g_accum=False,
                    mul_eng="gpsimd", batch_recip=False, alt_copies=False,
                    split_in=True, q2c_inline=False, o4_split=True,
                    ps_tr_bufs=2, ct_eng="dve", early_cout=False,
                    ps_s_bufs=2, ps_mm2_bufs=2, fine_tiles=True, fine_c=True)

_BUILT = None


def _build(reps=1, timing_mode=False, opts=None):
    import concourse.tile as tile
    from concourse import bacc, mybir
    from concourse.masks import make_identity

    o = dict(DEFAULT_OPTS)
    if opts:
        o.update(opts)

    f32 = mybir.dt.float32
    f32r = mybir.dt.float32r
    bf16 = mybir.dt.bfloat16
    AF = mybir.ActivationFunctionType
    AX = mybir.AxisListType
    ALU = mybir.AluOpType

    nc = bacc.Bacc("TRN2", target_bir_lowering=False, debug=False,
                   num_devices=N_CORES)
    c_d = nc.dram_tensor("c", [T, D], f32, kind="ExternalInput").ap()
    q_d = nc.dram_tensor("q", [I, D], f32, kind="ExternalInput").ap()
    wc_d = nc.dram_tensor("wc", [D], f32, kind="ExternalInput").ap()
    wq_d = nc.dram_tensor("wq", [D], f32, kind="ExternalInput").ap()
    wcq_d = nc.dram_tensor("wcq", [D], f32, kind="ExternalInput").ap()
    out_kind = "Internal" if timing_mode else "ExternalOutput"
    out_d = nc.dram_tensor("out", [T, 4 * D], f32, kind=out_kind).ap()
    tick_d = (nc.dram_tensor("tick", [1, 1], f32, kind="ExternalOutput").ap()
              if timing_mode else None)

    out_eng = {"sync": nc.sync, "scalar": nc.scalar}[o["out_ring"]]

    with tile.TileContext(nc) as tc:
        with (
            tc.tile_pool(name="const", bufs=1) as constp,
            tc.tile_pool(name="big", bufs=1) as bigp,
            tc.tile_pool(name="work", bufs=o["bufs_work"]) as workp,
            tc.tile_pool(name="outp", bufs=o["bufs_out"]) as outp,
            tc.tile_pool(name="ps_tr", bufs=o["ps_tr_bufs"],
                         space="PSUM") as ps_tr,
            tc.tile_pool(name="ps_acc", bufs=1, space="PSUM") as ps_acc,
            tc.tile_pool(name="ps_s", bufs=o["ps_s_bufs"],
                         space="PSUM") as ps_s,
            tc.tile_pool(name="ps_mm2", bufs=o["ps_mm2_bufs"],
                         space="PSUM") as ps_mm2,
        ):
            for _rep in range(reps):
                # ---------------- phase 0 -----------------------------------
                ident_f = constp.tile([128, 128], f32, tag="idf")
                make_identity(nc, ident_f[:])
                ident_b = constp.tile([128, 128], bf16, tag="idb")
                make_identity(nc, ident_b[:])
                ones_row_f = constp.tile([1, 128], f32, tag="ones_row_f")
                nc.vector.memset(ones_row_f[:], 1.0)
                ones_row = constp.tile([1, 128], f32r, tag="ones_row")
                nc.vector.tensor_copy(ones_row[:], ones_row_f[:])
                ones_col = constp.tile([128, 1], f32, tag="ones_col")
                nc.vector.memset(ones_col[:], 1.0)

                wcq_col = constp.tile([128, KC], f32, tag="wcq_col")
                nc.sync.dma_start(wcq_col[:],
                                  wcq_d.rearrange("(a b) -> b a", b=128))
                wc_col = constp.tile([128, KC], f32, tag="wc_col")
                nc.sync.dma_start(wc_col[:],
                                  wc_d.rearrange("(a b) -> b a", b=128))
                wq_col = constp.tile([128, KC], f32, tag="wq_col")
                nc.sync.dma_start(wq_col[:],
                                  wq_d.rearrange("(a b) -> b a", b=128))

                q_sb = bigp.tile([128, KC, D], f32, tag="q_sb")
                if o["memset_in"]:
                    nc.gpsimd.memset(q_sb[:], 0.01)
                elif o["contig_in"]:
                    nc.sync.dma_start(
                        q_sb[:], q_d.rearrange("(p k) d -> p k d", k=KC))
                else:
                    nc.sync.dma_start(
                        q_sb[:], q_d.rearrange("(k p) d -> p k d", p=128))
                q_bf = bigp.tile([128, KC, D], bf16, tag="q_bf")
                nc.vector.tensor_copy(q_bf[:], q_sb[:])

                c_sb = []
                if o["memset_in"]:
                    for jj in range(4):
                        t_ = bigp.tile([128, 4, D], f32, tag=f"c_sb{jj}")
                        nc.gpsimd.memset(t_[:], 0.02)
                        c_sb.append(t_)
                elif o["contig_in"] and o["fine_c"]:
                    crs = c_d.rearrange("(p j) d -> p j d", j=TT)
                    c_fine = []
                    for _j in range(TT):
                        cf = bigp.tile([128, D], f32, tag=f"cin{_j}")
                        nc.sync.dma_start(cf[:], crs[:, _j, :])
                        c_fine.append(cf)
                elif o["contig_in"]:
                    crs = c_d.rearrange("(p j) d -> p j d", j=TT)
                    if o["split_in"]:
                        for jj in range(4):
                            t_ = bigp.tile([128, 4, D], f32, tag=f"c_sb{jj}")
                            for jr in range(4):
                                nc.sync.dma_start(
                                    t_[:, jr:jr + 1, :],
                                    crs[:, 4 * jj + jr:4 * jj + jr + 1, :])
                            c_sb.append(t_)
                    else:
                        for jj in range(4):
                            t_ = bigp.tile([128, 4, D], f32, tag=f"c_sb{jj}")
                            nc.sync.dma_start(t_[:],
                                              crs[:, 4 * jj:4 * jj + 4, :])
                            c_sb.append(t_)
                else:
                    for jj in range(4):
                        t_ = bigp.tile([128, 4, D], f32, tag=f"c_sb{jj}")
                        nc.sync.dma_start(
                            t_[:],
                            c_d[jj * 512:(jj + 1) * 512, :].rearrange(
                                "(j p) d -> p j d", p=128))
                        c_sb.append(t_)

                if o["contig_in"]:
                    ors = out_d.rearrange("(p j) w -> p j w", j=TT)

                    def out_ap(j, sl):
                        return ors[:, j, sl]
                else:
                    def out_ap(j, sl):
                        return out_d[j * 128:(j + 1) * 128, sl]

                def c_tile(j):
                    if o["contig_in"] and o["fine_c"]:
                        return c_fine[j]
                    jj_, jr_ = divmod(j, 4)
                    return c_sb[jj_][:, jr_]

                _out_n = [0]

                def out_dma(j, sl, src):
                    if o["skip_out"]:
                        return
                    _out_n[0] += 1
                    eng = (nc.scalar if (o["out_split"] and _out_n[0] % 2)
                           else out_eng)
                    eng.dma_start(out_ap(j, sl), src)

                if o["dma_pair"]:
                    for j in range(TT):
                        jj, jr = divmod(j, 4)
                        cj = c_sb[jj][:, jr]
                        out_dma(j, slice(0, 512), cj[:])
                        out_dma(j, slice(512, 2048),
                                c_sb[jj][:].rearrange("p a d -> p (a d)")
                                [:, 0:1536])
                    continue

                def copy_op(dst, src):
                    if o["act_copies"] == "dve":
                        nc.vector.tensor_copy(dst, src)
                    else:
                        nc.scalar.copy(dst, src)

                if o["early_cout"] and not o["dma_pair"]:
                    for j in range(TT):
                        jj, jr = divmod(j, 4)
                        out_dma(j, slice(0, 512), c_sb[jj][:, jr])

                # Q^T, qa = Q^T * wcq + wc
                qt = bigp.tile([128, KC, I], f32, tag="qt")
                qa = bigp.tile([128, KC, I], f32r, tag="qa")
                for k in range(KC):
                    pt = ps_tr.tile([128, I], f32, tag="ps_tr")
                    for ik in range(KC):
                        nc.tensor.transpose(
                            pt[:, ik * 128:(ik + 1) * 128],
                            q_sb[:, ik, k * 128:(k + 1) * 128],
                            ident_f[:])
                    copy_op(qt[:, k], pt[:])
                    nc.vector.tensor_scalar(
                        qa[:, k], pt[:], wcq_col[:, k:k + 1],
                        wc_col[:, k:k + 1], op0=ALU.mult, op1=ALU.add)

                # sq_row[1, I] = w_q^T Q^T
                ps_sq = ps_s.tile([1, I], f32, tag="ps_s")
                for k in range(KC):
                    nc.tensor.matmul(ps_sq[:], wq_col[:, k:k + 1], qt[:, k],
                                     start=(k == 0), stop=(k == KC - 1))
                sq_row = constp.tile([1, I], f32r, tag="sq_row")
                copy_op(sq_row[:], ps_sq[:])

                scratch1 = constp.tile([1, 1], f32, tag="scratch1")
                g = constp.tile([128, TT], f32, tag="g")
                if o["q2c_inline"]:
                    psq2c = ps_acc.tile([1, D], f32, tag="ps_q2c")
                    psZ = ps_acc.tile([1, 1], f32, tag="ps_Z")
                mhat = constp.tile([128, TT], f32, tag="mhat")
                r_col = constp.tile([128, TT], f32, tag="r_col")
                rinv = constp.tile([128, TT], f32, tag="rinv")
                if o["fine_tiles"]:
                    et_tiles = []
                    for _j in range(TT):
                        et_j = bigp.tile([128, KC, 128], bf16,
                                         tag=f"et{_j}")
                        et_tiles.append(et_j)
                    r_tiles = []
                    ri_tiles = []
                    for _j in range(TT):
                        r_j = bigp.tile([128, 1], f32, tag=f"r{_j}")
                        r_tiles.append(r_j)
                        ri_j = bigp.tile([128, 1], f32, tag=f"ri{_j}")
                        ri_tiles.append(ri_j)
                else:
                    et = bigp.tile([128, KC, T], bf16, tag="et")

                # ---------------- phase 1: per row-tile ----------------------
                def do_mm2_epilogue(j, q2c_bc):
                    cj = c_tile(j)
                    pc = ps_mm2.tile([128, D], f32, tag="ps_mm2")
                    for ik in range(KC):
                        lhs_mm2 = (et_tiles[j][:, ik, :] if o["fine_tiles"]
                                   else et[:, ik, j * 128:(j + 1) * 128])
                        nc.tensor.matmul(pc[:], lhs_mm2, q_bf[:, ik],
                                         start=(ik == 0), stop=(ik == KC - 1))
                    if q2c_bc is None:
                        o_t = outp.tile([128, 1024], f32, tag="o23")
                        if o["act_copies"] == "dve":
                            nc.vector.tensor_scalar_mul(o_t[:, 0:512], pc[:],
                                                        (ri_tiles[j][:] if o["fine_tiles"] else rinv[:, j:j + 1]))
                        else:
                            nc.scalar.mul(o_t[:, 0:512], pc[:],
                                          (ri_tiles[j][:] if o["fine_tiles"]
                                           else rinv[:, j:j + 1]))
                        mul_e = (nc.gpsimd if o["mul_eng"] == "gpsimd"
                                 else nc.vector)
                        mul_e.tensor_mul(o_t[:, 512:1024], cj[:],
                                         o_t[:, 0:512])
                        if o["dup_dve"]:
                            nc.vector.tensor_mul(o_t[:, 512:1024], cj[:],
                                                 o_t[:, 0:512])
                        out_dma(j, slice(512, 1536), o_t[:])
                    else:
                        o_t = outp.tile([128, 1536], f32, tag="o234")
                        if o["act_copies"] == "dve":
                            nc.vector.tensor_scalar_mul(o_t[:, 0:512], pc[:],
                                                        (ri_tiles[j][:] if o["fine_tiles"] else rinv[:, j:j + 1]))
                        else:
                            nc.scalar.mul(o_t[:, 0:512], pc[:],
                                          (ri_tiles[j][:] if o["fine_tiles"]
                                           else rinv[:, j:j + 1]))
                        nc.vector.tensor_mul(o_t[:, 512:1024], cj[:],
                                             o_t[:, 0:512])
                        nc.vector.tensor_mul(o_t[:, 1024:1536], cj[:],
                                             q2c_bc[:])
                        out_dma(j, slice(512, 2048), o_t[:])

                for j in range(TT):
                    cj = c_tile(j)  # [128, 512] fp32

                    # C^T for this tile
                    pt = ps_tr.tile([128, 512], f32, tag="ps_tr")
                    for k in range(KC):
                        nc.tensor.transpose(
                            pt[:, k * 128:(k + 1) * 128],
                            cj[:, k * 128:(k + 1) * 128], ident_f[:])
                    ct = workp.tile([128, 512], f32r, tag="ct")
                    if o["ct_eng"] == "act" or (o["alt_copies"] and j % 2 == 0):
                        nc.scalar.copy(ct[:], pt[:])
                    else:
                        nc.vector.tensor_copy(ct[:], pt[:])
                    if o["dup_dve"]:
                        nc.vector.tensor_copy(ct[:], pt[:])

                    # mm1: s' = c @ qa + 1*sq
                    ps = ps_s.tile([128, I], f32, tag="ps_s")
                    if o["dup_pe"]:
                        for k in range(KC):
                            nc.tensor.matmul(
                                ps[:], ct[:, k * 128:(k + 1) * 128],
                                qa[:, k], start=(k == 0), stop=False,
                                skip_group_check=True)
                        for k in range(KC):
                            nc.tensor.matmul(
                                ps[:], ct[:, k * 128:(k + 1) * 128],
                                qa[:, k], start=(k == 0), stop=False,
                                skip_group_check=True)
                    else:
                        for k in range(KC):
                            nc.tensor.matmul(
                                ps[:], ct[:, k * 128:(k + 1) * 128],
                                qa[:, k], start=(k == 0), stop=False)
                    nc.tensor.matmul(ps[:], ones_row[:], sq_row[:],
                                     start=False, stop=True)

                    nc.vector.reduce_max(mhat[:, j:j + 1], ps[:], axis=AX.X)

                    e_tile = workp.tile([128, I], bf16, tag="e")
                    r_dst = (r_tiles[j][:] if o["fine_tiles"]
                             else r_col[:, j:j + 1])
                    if o["exp_accum"]:
                        nc.scalar.activation(e_tile[:], ps[:], AF.Exp,
                                             accum_out=r_dst)
                    else:
                        nc.scalar.activation(e_tile[:], ps[:], AF.Exp)
                        nc.vector.reduce_sum(r_dst, e_tile[:], axis=AX.X)
                    if o["dup_act"]:
                        nc.scalar.activation(e_tile[:], ps[:], AF.Exp,
                                             accum_out=r_col[:, j:j + 1])
                    if o["fine_tiles"]:
                        nc.vector.reciprocal(ri_tiles[j][:], r_tiles[j][:])
                    elif o["batch_recip"]:
                        if j % 4 == 3:
                            nc.vector.reciprocal(rinv[:, j - 3:j + 1],
                                                 r_col[:, j - 3:j + 1])
                    else:
                        nc.vector.reciprocal(rinv[:, j:j + 1],
                                             r_col[:, j:j + 1])

                    # E^T into et[:, ik, j*128:...]
                    pe = ps_tr.tile([128, 512], bf16, tag="ps_tr")
                    for ik in range(KC):
                        nc.tensor.transpose(
                            pe[:, ik * 128:(ik + 1) * 128],
                            e_tile[:, ik * 128:(ik + 1) * 128], ident_b[:])
                    et_dst = (et_tiles[j][:] if o["fine_tiles"]
                              else et[:, :, j * 128:(j + 1) * 128])
                    if o["alt_copies"] and j % 2 == 1:
                        nc.vector.tensor_copy(
                            et_dst, pe[:].rearrange("p (a b) -> p a b", a=KC))
                    else:
                        copy_op(et_dst,
                                pe[:].rearrange("p (a b) -> p a b", a=KC))

                    for _b in range(o["bloat"]):
                        nc.vector.memset(scratch1[0:1, 0:1], 0.0)

                    if o["q2c_inline"]:
                        nc.scalar.activation(g[:, j:j + 1], mhat[:, j:j + 1],
                                             AF.Exp)
                        nc.tensor.matmul(psq2c[:], g[:, j:j + 1], cj[:],
                                         start=(j == 0), stop=(j == TT - 1),
                                         skip_group_check=True)
                        nc.tensor.matmul(psZ[:], g[:, j:j + 1], ones_col[:],
                                         start=(j == 0), stop=(j == TT - 1),
                                         skip_group_check=True)

                    # c block can go out as soon as loaded
                    if not o["early_cout"]:
                        out_dma(j, slice(0, 512), cj[:])

                    if not o["two_pass"]:
                        do_mm2_epilogue(j, None)

                # ---------------- phase 2: q2c -------------------------------
                if not o["q2c_inline"]:
                    gsum = constp.tile([128, 1], f32, tag="gsum")
                    if o["g_accum"]:
                        nc.scalar.activation(g[:], mhat[:], AF.Exp,
                                             accum_out=gsum[:])
                    else:
                        nc.scalar.activation(g[:], mhat[:], AF.Exp)
                        nc.vector.reduce_sum(gsum[:], g[:], axis=AX.X)
                    psZ = ps_s.tile([1, 1], f32, tag="ps_s")
                    nc.tensor.matmul(psZ[:], ones_col[:], gsum[:],
                                     start=True, stop=True)
                    psq2c = ps_s.tile([1, D], f32, tag="ps_s")
                    for j in range(TT):
                        nc.tensor.matmul(psq2c[:], g[:, j:j + 1], c_tile(j),
                                         start=(j == 0), stop=(j == TT - 1))
                Zinv = constp.tile([1, 1], f32, tag="Zinv")
                nc.vector.reciprocal(Zinv[:], psZ[:])
                q2c_row = constp.tile([1, D], f32, tag="q2c_row")
                nc.vector.tensor_scalar_mul(q2c_row[:], psq2c[:], Zinv[:])

                psbc = ps_s.tile([128, D], f32, tag="ps_s")
                nc.tensor.matmul(psbc[:], ones_row_f[:], q2c_row[:],
                                 start=True, stop=True)
                q2c_bc = constp.tile([128, D], f32, tag="q2c_bc")
                copy_op(q2c_bc[:], psbc[:])

                # ---------------- phase 3 ------------------------------------
                if o["two_pass"]:
                    for j in range(TT):
                        do_mm2_epilogue(j, q2c_bc)
                else:
                    for j in range(TT):
                        jj, jr = divmod(j, 4)
                        if o["o4_split"]:
                            mul_e4 = nc.gpsimd if j % 2 else nc.vector
                        else:
                            mul_e4 = (nc.gpsimd if o["mul_eng"] == "gpsimd"
                                      else nc.vector)
                        o4 = outp.tile([128, D], f32, tag="o4")
                        mul_e4.tensor_mul(o4[:], c_tile(j), q2c_bc[:])
                        out_dma(j, slice(1536, 2048), o4[:])

        if timing_mode:
            with tc.tile_pool(name="tickp", bufs=1) as tickp:
                tk = tickp.tile([1, 1], f32, tag="tick")
                nc.vector.memset(tk[:], 1.0)
                nc.sync.dma_start(tick_d[:], tk[:])

    nc.compile()
    return nc


def _get_built():
    global _BUILT
    if _BUILT is None:
        _BUILT = _build()
    return _BUILT


def kernel(c, q, w_c, b_c, w_q, b_q, w_cq, b_cq):
    """Full inputs in, full output out. Data-parallel over batch on 8 cores.

    Biases cancel mathematically (softmax shift invariance), so b_* are
    accepted but unused.
    """
    from concourse import bass_utils

    nc = _get_built()
    c = np.ascontiguousarray(np.asarray(c, dtype=np.float32))
    q = np.ascontiguousarray(np.asarray(q, dtype=np.float32))
    wc = np.ascontiguousarray(np.asarray(w_c, dtype=np.float32))
    wq = np.ascontiguousarray(np.asarray(w_q, dtype=np.float32))
    wcq = np.ascontiguousarray(np.asarray(w_cq, dtype=np.float32))

    in_maps = [
        {"c": c[b], "q": q[b], "wc": wc, "wq": wq, "wcq": wcq}
        for b in range(N_CORES)
    ]
    res = bass_utils.run_bass_kernel_spmd(
        nc, in_maps, core_ids=list(range(N_CORES)))
    return np.stack([res.results[b]["out"] for b in range(N_CORES)])



# revision 20
# speedup vs baseline: 1.1970x; 1.1970x over previous
"""Trainium2 Bass kernel for nn_AttentionFlow (BiDAF-style attention flow).

Math (per batch b, biases cancel):
  s[t,i]   = <c_t,w_c> + <q_i,w_q> + <c_t*q_i, w_cq>  (+ biases)
  a        = softmax_i(s)          -> c2q = a @ q
  beta     = softmax_t(max_i s)    -> q2c = beta^T c
  out      = [c | c2q | c*c2q | c*q2c]

Design: everything is computed in the TRANSPOSED score domain.
  s'^T[i,t] = qa^T @ c^T + sq (x) 1        (qa[d,i] = q^T*w_cq + w_c)
  e^T = exp(s'^T)   [i on partitions, t free]  -- born as mm2's lhsT,
  c2q[t,d] = (e^T)^T @ q                       -- natural output layout,
  r[t]     = (e^T)^T @ 1                       -- N=1 matmuls, shared weights,
  g[t]     = max_i e^T[i,t] = exp(max_i s')    -- bf16 DVE partition tree,
  beta     = g/sum(g), q2c = beta^T c via PE after a tiny row->col transpose.

This removes all 64 E^T PE transposes of the naive layout; the only PE
transposes are c^T (64), q^T (16) and the 4x [4,128] g-row flips.
t and i orderings are arbitrary (softmax/contractions are order-invariant,
outputs re-addressed by AP), so row->partition maps are chosen for DMA
contiguity: t = p*16 + j, i = 4*p + k.

Sharding: data-parallel over batch, one batch element per NeuronCore (8).
"""

import numpy as np

N_CORES = 8
T, I, D = 2048, 512, 512
TT = T // 128   # 16 row tiles
KC = 4          # 128-chunks of D
IC = 4          # 128-chunks of I
NG = 4          # t-groups of 512 rows (4 tiles each)

DEFAULT_OPTS = dict(
    bufs_work=3, bufs_out=3, ps_tr_bufs=2, ps_s_bufs=2, ps_mm2_bufs=3,
    ct_acts=10,      # how many of the 16 ct copies go on ACT (rest DVE)
    o2_acts=10,      # how many of the 16 o2 scales go on ACT (rest DVE)
    o3_dve=8,        # how many of the 16 o3 muls go on DVE (rest GPSIMD)
    o4_dve=12,       # how many of the 16 o4 muls go on DVE (rest GPSIMD)
    skip_out=False,
)

_BUILT = None


def _build(reps=1, timing_mode=False, opts=None):
    import concourse.tile as tile
    from concourse import bacc, bass_isa, mybir
    from concourse.masks import make_identity

    o = dict(DEFAULT_OPTS)
    if opts:
        o.update(opts)

    f32 = mybir.dt.float32
    f32r = mybir.dt.float32r
    bf16 = mybir.dt.bfloat16
    AF = mybir.ActivationFunctionType
    AX = mybir.AxisListType
    ALU = mybir.AluOpType

    nc = bacc.Bacc("TRN2", target_bir_lowering=False, debug=False,
                   num_devices=N_CORES)
    c_d = nc.dram_tensor("c", [T, D], f32, kind="ExternalInput").ap()
    q_d = nc.dram_tensor("q", [I, D], f32, kind="ExternalInput").ap()
    wc_d = nc.dram_tensor("wc", [D], f32, kind="ExternalInput").ap()
    wq_d = nc.dram_tensor("wq", [D], f32, kind="ExternalInput").ap()
    wcq_d = nc.dram_tensor("wcq", [D], f32, kind="ExternalInput").ap()
    out_kind = "Internal" if timing_mode else "ExternalOutput"
    out_d = nc.dram_tensor("out", [T, 4 * D], f32, kind=out_kind).ap()
    tick_d = (nc.dram_tensor("tick", [1, 1], f32, kind="ExternalOutput").ap()
              if timing_mode else None)

    with tile.TileContext(nc) as tc:
        with (
            tc.tile_pool(name="const", bufs=1) as constp,
            tc.tile_pool(name="big", bufs=1) as bigp,
            tc.tile_pool(name="work", bufs=o["bufs_work"]) as workp,
            tc.tile_pool(name="outp", bufs=o["bufs_out"]) as outp,
            tc.tile_pool(name="ps_tr", bufs=o["ps_tr_bufs"],
                         space="PSUM") as ps_tr,
            tc.tile_pool(name="ps_s", bufs=o["ps_s_bufs"],
                         space="PSUM") as ps_s,
            tc.tile_pool(name="ps_mm2", bufs=o["ps_mm2_bufs"],
                         space="PSUM") as ps_mm2,
            tc.tile_pool(name="ps_q2c", bufs=1, space="PSUM") as ps_q2c,
        ):
            for _rep in range(reps):
                # ---------------- constants --------------------------------
                ident_f = constp.tile([128, 128], f32, tag="idf")
                make_identity(nc, ident_f[:])
                ones_row_f5 = constp.tile([1, 512], f32, tag="ones_row_f5")
                nc.vector.memset(ones_row_f5[:], 1.0)
                ones_row_t = constp.tile([1, 512], f32r, tag="ones_row_t")
                nc.vector.tensor_copy(ones_row_t[:], ones_row_f5[:])
                ones_row_f = constp.tile([1, 128], f32, tag="ones_row_f")
                nc.vector.memset(ones_row_f[:], 1.0)
                ones_col_b = constp.tile([128, 1], bf16, tag="ones_col_b")
                nc.vector.memset(ones_col_b[:], 1.0)
                ones_col_f = constp.tile([128, 1], f32, tag="ones_col_f")
                nc.vector.memset(ones_col_f[:], 1.0)

                wcq_col = constp.tile([128, KC], f32, tag="wcq_col")
                nc.sync.dma_start(wcq_col[:],
                                  wcq_d.rearrange("(a b) -> b a", b=128))
                wc_col = constp.tile([128, KC], f32, tag="wc_col")
                nc.sync.dma_start(wc_col[:],
                                  wc_d.rearrange("(a b) -> b a", b=128))
                wq_col = constp.tile([128, KC], f32, tag="wq_col")
                nc.sync.dma_start(wq_col[:],
                                  wq_d.rearrange("(a b) -> b a", b=128))

                # ---------------- q path -----------------------------------
                # i-map: partition p, chunk k -> i = 4*p + k
                q_sb = bigp.tile([128, IC, D], f32, tag="q_sb")
                nc.sync.dma_start(q_sb[:],
                                  q_d.rearrange("(p k) d -> p k d", k=IC))
                q_bf = bigp.tile([128, IC, D], bf16, tag="q_bf")
                nc.vector.tensor_copy(q_bf[:], q_sb[:])

                # q^T (i-free layout ii = 128*ik + p), qa = q^T*wcq + wc
                qt = bigp.tile([128, KC, I], f32, tag="qt")
                qa = bigp.tile([128, KC, I], f32r, tag="qa")
                for k in range(KC):
                    pt = ps_tr.tile([128, I], f32, tag="ps_tr")
                    for ik in range(IC):
                        nc.tensor.transpose(
                            pt[:, ik * 128:(ik + 1) * 128],
                            q_sb[:, ik, k * 128:(k + 1) * 128],
                            ident_f[:])
                    nc.scalar.copy(qt[:, k], pt[:])
                    nc.vector.tensor_scalar(
                        qa[:, k], pt[:], wcq_col[:, k:k + 1],
                        wc_col[:, k:k + 1], op0=ALU.mult, op1=ALU.add)

                # sq_row[1, I] = w_q^T @ q^T   (ii layout)
                ps_sq = ps_q2c.tile([1, I], f32, tag="ps_q2c")
                for k in range(KC):
                    nc.tensor.matmul(ps_sq[:], wq_col[:, k:k + 1], qt[:, k],
                                     start=(k == 0), stop=(k == KC - 1))
                sq_row = constp.tile([1, I], f32r, tag="sq_row")
                nc.vector.tensor_copy(sq_row[:], ps_sq[:])

                # ---------------- main tiles -------------------------------
                crs = c_d.rearrange("(p j) d -> p j d", j=TT)
                ors = out_d.rearrange("(p j) w -> p j w", j=TT)

                c_fine = [bigp.tile([128, D], f32, tag=f"cin{j}",
                                    name=f"cin{j}") for j in range(TT)]
                ct_g = [bigp.tile([128, KC, 512], f32r, tag=f"ct{g}",
                                  name=f"ct{g}") for g in range(NG)]
                et_g = [bigp.tile([128, IC, 512], bf16, tag=f"et{g}",
                                  name=f"et{g}") for g in range(NG)]
                rinv_g = [bigp.tile([128, NG], f32, tag=f"rinv{g}",
                                    name=f"rinv{g}") for g in range(NG)]
                gm_g = [bigp.tile([128, 512], f32, tag=f"gm{g}",
                                  name=f"gm{g}") for g in range(NG)]

                _n = dict(ct=0, o2=0, o3=0, o4=0, odma=0)

                def out_dma(j, sl, src):
                    if o["skip_out"]:
                        return
                    _n["odma"] += 1
                    eng = nc.scalar if _n["odma"] % 2 else nc.sync
                    eng.dma_start(ors[:, j, sl], src)

                # ---------------- per-group pipeline -----------------------
                for g in range(NG):
                    # load 4 c tiles, echo out as o1
                    for b in range(4):
                        j = 4 * g + b
                        nc.sync.dma_start(c_fine[j][:], crs[:, j, :])
                        out_dma(j, slice(0, 512), c_fine[j][:])

                    # c^T for this group: ct_g[g][dk, k, 128*b + pc]
                    for k in range(KC):
                        pt = ps_tr.tile([128, 512], f32, tag="ps_tr")
                        for b in range(4):
                            nc.tensor.transpose(
                                pt[:, b * 128:(b + 1) * 128],
                                c_fine[4 * g + b][:, k * 128:(k + 1) * 128],
                                ident_f[:])
                        _n["ct"] += 1
                        if _n["ct"] <= o["ct_acts"]:
                            nc.scalar.copy(ct_g[g][:, k, :], pt[:])
                        else:
                            nc.vector.tensor_copy(ct_g[g][:, k, :], pt[:])

                    # mm1: s'^T[im, t] = sum_k qa[k,im]^T @ ct + sq (x) ones
                    for m in range(IC):
                        ps = ps_s.tile([128, 512], f32, tag="ps_s")
                        for k in range(KC):
                            nc.tensor.matmul(
                                ps[:], qa[:, k, m * 128:(m + 1) * 128],
                                ct_g[g][:, k, :],
                                start=(k == 0), stop=False)
                        nc.tensor.matmul(
                            ps[:], sq_row[0:1, m * 128:(m + 1) * 128],
                            ones_row_t[:], start=False, stop=True)
                        nc.scalar.activation(et_g[g][:, m, :], ps[:], AF.Exp)

                    # g-row: gmax over i = chunk-max (DVE) + partition
                    # all-reduce max (GPSIMD daisy chain)
                    tr0 = workp.tile([128, 512], bf16, tag="tr0")
                    tr1 = workp.tile([128, 512], bf16, tag="tr1")
                    nc.vector.tensor_tensor(tr0[:], et_g[g][:, 0, :],
                                            et_g[g][:, 1, :], op=ALU.max)
                    nc.vector.tensor_tensor(tr1[:], et_g[g][:, 2, :],
                                            et_g[g][:, 3, :], op=ALU.max)
                    nc.vector.tensor_tensor(tr0[:], tr0[:], tr1[:],
                                            op=ALU.max)
                    nc.gpsimd.partition_all_reduce(
                        gm_g[g][:], tr0[:], 128, bass_isa.ReduceOp.max)

                    # mm2 + row sums + o2/o3 per tile
                    pcs = []
                    rps = None
                    for b in range(4):
                        if b % 2 == 0:
                            rps = ps_s.tile([128, 2], f32, tag="ps_s",
                                            name="rps")
                        pc = ps_mm2.tile([128, 512], f32, tag="ps_mm2")
                        pcs.append(pc)
                        for m in range(IC):
                            lhs = et_g[g][:, m, b * 128:(b + 1) * 128]
                            nc.tensor.matmul(pc[:], lhs, q_bf[:, m, :],
                                             start=(m == 0), stop=(m == IC - 1),
                                             skip_group_check=True)
                            nc.tensor.matmul(rps[:, b % 2:b % 2 + 1], lhs,
                                             ones_col_b[:],
                                             start=(m == 0), stop=(m == IC - 1),
                                             skip_group_check=True)
                        if b % 2 == 1:
                            nc.vector.reciprocal(rinv_g[g][:, b - 1:b + 1],
                                                 rps[:])
                    for b in range(4):
                        j = 4 * g + b
                        pc = pcs[b]
                        o_t = outp.tile([128, 1024], f32, tag="o23")
                        _n["o2"] += 1
                        if _n["o2"] <= o["o2_acts"]:
                            nc.scalar.mul(o_t[:, 0:512], pc[:],
                                          rinv_g[g][:, b:b + 1])
                        else:
                            nc.vector.tensor_scalar_mul(o_t[:, 0:512], pc[:],
                                                        rinv_g[g][:, b:b + 1])
                        _n["o3"] += 1
                        o3e = (nc.vector if _n["o3"] <= o["o3_dve"]
                               else nc.gpsimd)
                        o3e.tensor_mul(o_t[:, 512:1024], c_fine[j][:],
                                       o_t[:, 0:512])
                        out_dma(j, slice(512, 1536), o_t[:])

                # ---------------- q2c (beta softmax over t) ----------------
                # gm_g rows (identical across partitions) -> g columns per
                # tile via skinny K=1 PE transposes: mcol[:, j] for j=4g+b.
                pmc = ps_tr.tile([128, 16], f32, tag="ps_tr")
                for j in range(TT):
                    g_, b_ = divmod(j, 4)
                    nc.tensor.transpose(
                        pmc[:, j:j + 1],
                        gm_g[g_][0:1, b_ * 128:(b_ + 1) * 128],
                        ident_f[0:1, 0:1])
                mcol = constp.tile([128, 16], f32, tag="mcol")
                nc.vector.tensor_copy(mcol[:], pmc[:])
                zcol = constp.tile([128, 1], f32, tag="zcol")
                nc.vector.reduce_sum(zcol[:], mcol[:], axis=AX.X)
                psZ = ps_q2c.tile([1, 1], f32, tag="ps_q2c")
                nc.tensor.matmul(psZ[:], zcol[:], ones_col_f[:],
                                 start=True, stop=True)
                Zinv = constp.tile([1, 1], f32, tag="Zinv")
                nc.vector.reciprocal(Zinv[:], psZ[:])
                psq2c = ps_q2c.tile([1, D], f32, tag="ps_q2c")
                for j in range(TT):
                    nc.tensor.matmul(psq2c[:], mcol[:, j:j + 1],
                                     c_fine[j][:],
                                     start=(j == 0), stop=(j == TT - 1))
                q2c_row = constp.tile([1, D], f32, tag="q2c_row")
                nc.vector.tensor_scalar_mul(q2c_row[:], psq2c[:], Zinv[:])
                psbc = ps_q2c.tile([128, D], f32, tag="ps_q2c")
                nc.tensor.matmul(psbc[:], ones_row_f[:], q2c_row[:],
                                 start=True, stop=True)
                q2c_bc = constp.tile([128, D], f32, tag="q2c_bc")
                nc.scalar.copy(q2c_bc[:], psbc[:])

                # ---------------- o4 = c * q2c -----------------------------
                for j in range(TT):
                    o4 = outp.tile([128, 512], f32, tag="o4")
                    _n["o4"] += 1
                    eng = nc.vector if _n["o4"] <= o["o4_dve"] else nc.gpsimd
                    eng.tensor_mul(o4[:], c_fine[j][:], q2c_bc[:])
                    out_dma(j, slice(1536, 2048), o4[:])

        if timing_mode:
            with tc.tile_pool(name="tickp", bufs=1) as tickp:
                tk = tickp.tile([1, 1], f32, tag="tick")
                nc.vector.memset(tk[:], 1.0)
                nc.sync.dma_start(tick_d[:], tk[:])

    nc.compile()
    return nc


def _get_built():
    global _BUILT
    if _BUILT is None:
        _BUILT = _build()
    return _BUILT


def kernel(c, q, w_c, b_c, w_q, b_q, w_cq, b_cq):
    """Full inputs in, full output out. Data-parallel over batch on 8 cores.

    Biases cancel mathematically (softmax shift invariance), so b_* are
    accepted but unused.
    """
    from concourse import bass_utils

    nc = _get_built()
    c = np.ascontiguousarray(np.asarray(c, dtype=np.float32))
    q = np.ascontiguousarray(np.asarray(q, dtype=np.float32))
    wc = np.ascontiguousarray(np.asarray(w_c, dtype=np.float32))
    wq = np.ascontiguousarray(np.asarray(w_q, dtype=np.float32))
    wcq = np.ascontiguousarray(np.asarray(w_cq, dtype=np.float32))

    in_maps = [
        {"c": c[b], "q": q[b], "wc": wc, "wq": wq, "wcq": wcq}
        for b in range(N_CORES)
    ]
    res = bass_utils.run_bass_kernel_spmd(
        nc, in_maps, core_ids=list(range(N_CORES)))
    return np.stack([res.results[b]["out"] for b in range(N_CORES)])


# revision 49
# speedup vs baseline: 5.1727x; 4.3212x over previous
"""Trainium2 Bass kernel for nn_AttentionFlow (BiDAF-style attention flow).

Math (per batch b, biases cancel):
  s[t,i]   = <c_t,w_c> + <q_i,w_q> + <c_t*q_i, w_cq>  (+ biases)
  a        = softmax_i(s)          -> c2q = a @ q
  beta     = softmax_t(max_i s)    -> q2c = beta^T c
  out      = [c | c2q | c*c2q | c*q2c]

Design: everything is computed in the TRANSPOSED score domain.
  s'^T[i,t] = qa^T @ c^T + sq (x) 1        (qa[d,i] = q^T*w_cq + w_c)
  e^T = exp(s'^T)   [i on partitions, t free]  -- born as mm2's lhsT,
  c2q[t,d] = (e^T)^T @ q                       -- natural output layout,
  r[t]     = (e^T)^T @ 1                       -- N=1 matmuls, shared weights,
  g[t]     = max_i e^T[i,t] = exp(max_i s')    -- GPSIMD partition all-reduce,
  beta     = g/sum(g), q2c = beta^T c via PE with per-group accumulation.

This removes all 64 E^T PE transposes of the naive layout; the only PE
transposes are c^T (64, bf16), q^T (16, bf16) and 16 skinny g-column flips.
t and i orderings are arbitrary (softmax/contractions are order-invariant,
outputs re-addressed by AP), so row->partition maps are chosen for DMA
contiguity: t = p*16 + j, i = 4*p + k.

Sharding: data-parallel over batch, one batch element per NeuronCore (8).
"""

import numpy as np

N_CORES = 8
T, I, D = 2048, 512, 512
TT = T // 128   # 16 row tiles
KC = 4          # 128-chunks of D
IC = 4          # 128-chunks of I
NG = 4          # t-groups of 512 rows (4 tiles each)

DEFAULT_OPTS = dict(
    bufs_work=3, bufs_out=2, ps_tr_bufs=2, ps_s_bufs=2, ps_mm2_bufs=3,
    ct_acts=10,      # how many of the 16 ct copies go on ACT (rest DVE)
    o2_acts=10,      # how many of the 16 o2 scales go on ACT (rest DVE)
    o3_dve=8,        # how many of the 16 o3 muls go on DVE (rest GPSIMD)
    o4_dve=10,       # how many of the 16 o4 muls go on DVE (rest GPSIMD)
    skip_out=False,
)

_BUILT = None


def _build(reps=1, timing_mode=False, opts=None):
    import concourse.tile as tile
    from concourse import bacc, bass_isa, mybir
    from concourse.masks import make_identity

    o = dict(DEFAULT_OPTS)
    if opts:
        o.update(opts)

    f32 = mybir.dt.float32
    bf16 = mybir.dt.bfloat16
    AF = mybir.ActivationFunctionType
    AX = mybir.AxisListType
    ALU = mybir.AluOpType

    nc = bacc.Bacc("TRN2", target_bir_lowering=False, debug=False,
                   num_devices=N_CORES)
    c_d = nc.dram_tensor("c", [T, D], f32, kind="ExternalInput").ap()
    q_d = nc.dram_tensor("q", [I, D], f32, kind="ExternalInput").ap()
    wc_d = nc.dram_tensor("wc", [D], f32, kind="ExternalInput").ap()
    wq_d = nc.dram_tensor("wq", [D], f32, kind="ExternalInput").ap()
    wcq_d = nc.dram_tensor("wcq", [D], f32, kind="ExternalInput").ap()
    out_kind = "Internal" if timing_mode else "ExternalOutput"
    out_d = nc.dram_tensor("out", [T, 4 * D], f32, kind=out_kind).ap()
    tick_d = (nc.dram_tensor("tick", [1, 1], f32, kind="ExternalOutput").ap()
              if timing_mode else None)

    with tile.TileContext(nc) as tc:
        with (
            tc.tile_pool(name="const", bufs=1) as constp,
            tc.tile_pool(name="big", bufs=1) as bigp,
            tc.tile_pool(name="work", bufs=o["bufs_work"]) as workp,
            tc.tile_pool(name="outp", bufs=o["bufs_out"]) as outp,
            tc.tile_pool(name="ps_tr", bufs=o["ps_tr_bufs"],
                         space="PSUM") as ps_tr,
            tc.tile_pool(name="ps_s", bufs=o["ps_s_bufs"],
                         space="PSUM") as ps_s,
            tc.tile_pool(name="ps_mm2", bufs=o["ps_mm2_bufs"],
                         space="PSUM") as ps_mm2,
            tc.tile_pool(name="ps_q2c", bufs=1, space="PSUM") as ps_q2c,
        ):
            for _rep in range(reps):
                crs = c_d.rearrange("(p j) d -> p j d", j=TT)
                ors = out_d.rearrange("(p j) w -> p j w", j=TT)
                qrs = q_d.rearrange("(p k) d -> p k d", k=IC)

                # ---------------- input DMAs (head-latency ordered) --------
                # q d-chunk 0 first (unblocks q^T), then c group 0, then the
                # rest of q; weights ride the scalar queue.
                q_sb = bigp.tile([128, IC, D], f32, tag="q_sb")
                c_gb = [bigp.tile([128, 4, D], f32, tag=f"cg{g}",
                                  name=f"cg{g}") for g in range(NG)]
                for k in range(KC):
                    nc.sync.dma_start(q_sb[:, :, k * 128:(k + 1) * 128],
                                      qrs[:, :, k * 128:(k + 1) * 128])
                nc.sync.dma_start(c_gb[0][:], crs[:, 0:4, :])

                wcq_col = constp.tile([128, KC], f32, tag="wcq_col")
                nc.scalar.dma_start(wcq_col[:],
                                    wcq_d.rearrange("(a b) -> b a", b=128))
                wc_col = constp.tile([128, KC], f32, tag="wc_col")
                nc.scalar.dma_start(wc_col[:],
                                    wc_d.rearrange("(a b) -> b a", b=128))
                wq_col = constp.tile([128, KC], f32, tag="wq_col")
                nc.scalar.dma_start(wq_col[:],
                                    wq_d.rearrange("(a b) -> b a", b=128))

                # ---------------- constants --------------------------------
                ident_b = constp.tile([128, 128], bf16, tag="idb")
                make_identity(nc, ident_b[:])
                ones_row_b5 = constp.tile([1, 512], bf16, tag="ones_row_b5")
                nc.vector.memset(ones_row_b5[:], 1.0)
                ones_row_b = constp.tile([1, 128], bf16, tag="ones_row_b")
                nc.vector.memset(ones_row_b[:], 1.0)
                ones_col_b = constp.tile([128, 1], bf16, tag="ones_col_b")
                nc.vector.memset(ones_col_b[:], 1.0)

                # ---------------- q path (bf16) ----------------------------
                # i-map: partition p, chunk k -> i = 4*p + k
                q_bf = bigp.tile([128, IC, D], bf16, tag="q_bf")
                for k in range(KC):
                    nc.vector.tensor_copy(q_bf[:, :, k * 128:(k + 1) * 128],
                                          q_sb[:, :, k * 128:(k + 1) * 128])

                # q^T (i-free layout ii = 128*ik + p), qa = q^T*wcq + wc
                qt = bigp.tile([128, KC, I], bf16, tag="qt")
                qa = bigp.tile([128, KC, I], bf16, tag="qa")
                for k in range(KC):
                    pt = ps_tr.tile([128, I], bf16, tag="ps_tr")
                    for ik in range(IC):
                        nc.tensor.transpose(
                            pt[:, ik * 128:(ik + 1) * 128],
                            q_bf[:, ik, k * 128:(k + 1) * 128],
                            ident_b[:])
                    nc.scalar.copy(qt[:, k], pt[:])
                    nc.vector.tensor_scalar(
                        qa[:, k], pt[:], wcq_col[:, k:k + 1],
                        wc_col[:, k:k + 1], op0=ALU.mult, op1=ALU.add)

                # sq_row[1, I] = w_q^T @ q^T   (ii layout)
                wq_b = constp.tile([128, KC], bf16, tag="wq_b")
                nc.vector.tensor_copy(wq_b[:], wq_col[:])
                ps_sq = ps_q2c.tile([1, I], f32, tag="ps_q2c")
                for k in range(KC):
                    nc.tensor.matmul(ps_sq[:], wq_b[:, k:k + 1], qt[:, k],
                                     start=(k == 0), stop=(k == KC - 1))
                sq_row = constp.tile([1, I], bf16, tag="sq_row")
                nc.vector.tensor_copy(sq_row[:], ps_sq[:])

                # ---------------- main tiles -------------------------------
                c_bf = [bigp.tile([128, 4, D], bf16, tag=f"cb{g}",
                                  name=f"cb{g}") for g in range(NG)]
                ct_g = [bigp.tile([128, KC, 512], bf16, tag=f"ct{g}",
                                  name=f"ct{g}") for g in range(NG)]
                et_g = [bigp.tile([128, IC, 512], bf16, tag=f"et{g}",
                                  name=f"et{g}") for g in range(NG)]
                rinv_g = [bigp.tile([128, NG], f32, tag=f"rinv{g}",
                                    name=f"rinv{g}") for g in range(NG)]
                gm_g = [bigp.tile([128, 512], bf16, tag=f"gm{g}",
                                  name=f"gm{g}") for g in range(NG)]
                mcol_g = [bigp.tile([128, 4], bf16, tag=f"mc{g}",
                                    name=f"mc{g}") for g in range(NG)]
                o23_g = [outp.tile([128, 4, 1024], f32, tag="o23",
                                   name=f"o23_{g}") for g in range(NG)]
                o4_g = [outp.tile([128, 4, 512], f32, tag="o4",
                                  name=f"o4_{g}") for g in range(NG)]
                zacc = constp.tile([128, 1], f32, tag="zacc")
                psq2c = [None]

                def c_fine(j):
                    g, b = divmod(j, 4)
                    return c_gb[g][:, b, :]

                _n = dict(ct=0, o2=0, o3=0, o4=0, odma=0)

                def out_dma(dst, src):
                    if o["skip_out"]:
                        return
                    _n["odma"] += 1
                    eng = nc.scalar if _n["odma"] % 2 else nc.sync
                    eng.dma_start(dst, src)

                # ---------------- pipeline stages --------------------------
                def phase1(g):
                    """loads + c^T + mm1 + exp + g-max + q2c partials."""
                    if g + 1 < NG:
                        nc.sync.dma_start(c_gb[g + 1][:],
                                          crs[:, 4 * (g + 1):4 * (g + 2), :])
                    nc.vector.tensor_copy(c_bf[g][:], c_gb[g][:])

                    # c^T for this group: ct_g[g][dk, k, 128*b + pc]
                    for k in range(KC):
                        pt = ps_tr.tile([128, 512], bf16, tag="ps_tr")
                        for b in range(4):
                            nc.tensor.transpose(
                                pt[:, b * 128:(b + 1) * 128],
                                c_bf[g][:, b, k * 128:(k + 1) * 128],
                                ident_b[:])
                        _n["ct"] += 1
                        if _n["ct"] <= o["ct_acts"]:
                            nc.scalar.copy(ct_g[g][:, k, :], pt[:])
                        else:
                            nc.vector.tensor_copy(ct_g[g][:, k, :], pt[:])

                    # mm1: s'^T[im, t] = sum_k qa[k,im]^T @ ct + sq (x) ones
                    for m in range(IC):
                        ps = ps_s.tile([128, 512], f32, tag="ps_s")
                        for k in range(KC):
                            nc.tensor.matmul(
                                ps[:], qa[:, k, m * 128:(m + 1) * 128],
                                ct_g[g][:, k, :],
                                start=(k == 0), stop=False,
                                skip_group_check=True)
                        nc.tensor.matmul(
                            ps[:], sq_row[0:1, m * 128:(m + 1) * 128],
                            ones_row_b5[:], start=False, stop=True,
                            skip_group_check=True)
                        nc.scalar.activation(et_g[g][:, m, :], ps[:], AF.Exp)

                    # g-row: gmax over i = chunk-max (DVE) + partition
                    # all-reduce max (GPSIMD daisy chain); gm rows identical
                    # across partitions.
                    tr0 = workp.tile([128, 512], bf16, tag="tr0")
                    tr1 = workp.tile([128, 512], bf16, tag="tr1")
                    nc.vector.tensor_tensor(tr0[:], et_g[g][:, 0, :],
                                            et_g[g][:, 1, :], op=ALU.max)
                    nc.vector.tensor_tensor(tr1[:], et_g[g][:, 2, :],
                                            et_g[g][:, 3, :], op=ALU.max)
                    nc.vector.tensor_tensor(tr0[:], tr0[:], tr1[:],
                                            op=ALU.max)
                    nc.gpsimd.partition_all_reduce(
                        gm_g[g][:], tr0[:], 128, bass_isa.ReduceOp.max)

                def q2c_partials(g):
                    """Deferred one stage so the PE queue never waits on
                    group g's partition all-reduce: g columns via skinny
                    transposes, then accumulate beta-weighted c and Z."""
                    # [128, 4, 2] so each bf16 column sits 4-byte aligned
                    pmc = ps_tr.tile([128, 4, 2], bf16, tag="ps_tr")
                    for b in range(4):
                        nc.tensor.transpose(
                            pmc[:, b, 0:1],
                            gm_g[g][0:1, b * 128:(b + 1) * 128],
                            ident_b[0:1, 0:1])
                    nc.vector.tensor_copy(mcol_g[g][:], pmc[:, :, 0])
                    if g == 0:
                        psq2c[0] = ps_q2c.tile([1, D], f32, tag="ps_q2c",
                                               name="psq2c")
                    for b in range(4):
                        nc.tensor.matmul(psq2c[0][:], mcol_g[g][:, b:b + 1],
                                         c_bf[g][:, b, :],
                                         start=(g == 0 and b == 0),
                                         stop=(g == NG - 1 and b == 3),
                                         skip_group_check=True)
                    # Z partial: every partition of gm_g holds the full
                    # g-row, so a free-dim sum gives the group Z everywhere.
                    zc = workp.tile([128, 1], f32, tag="zc")
                    nc.vector.reduce_sum(zc[:], gm_g[g][:], axis=AX.X)
                    if g == 0:
                        nc.vector.tensor_copy(zacc[:], zc[:])
                    else:
                        nc.vector.tensor_add(zacc[:], zacc[:], zc[:])

                def mm2_block(g, dma_fine=False):
                    """mm2 + row sums + o2/o3 + output DMA for group g."""
                    pcs = []
                    rps = None
                    for b in range(4):
                        if b % 2 == 0:
                            rps = ps_s.tile([128, 2], f32, tag="ps_s",
                                            name="rps")
                        pc = ps_mm2.tile([128, 512], f32, tag="ps_mm2")
                        pcs.append(pc)
                        for m in range(IC):
                            lhs = et_g[g][:, m, b * 128:(b + 1) * 128]
                            nc.tensor.matmul(pc[:], lhs, q_bf[:, m, :],
                                             start=(m == 0), stop=(m == IC - 1),
                                             skip_group_check=True)
                            nc.tensor.matmul(rps[:, b % 2:b % 2 + 1], lhs,
                                             ones_col_b[:],
                                             start=(m == 0), stop=(m == IC - 1),
                                             skip_group_check=True)
                        if b % 2 == 1:
                            nc.vector.reciprocal(rinv_g[g][:, b - 1:b + 1],
                                                 rps[:])
                    o_t = o23_g[g]
                    for b in range(4):
                        j = 4 * g + b
                        pc = pcs[b]
                        _n["o2"] += 1
                        if _n["o2"] <= o["o2_acts"]:
                            nc.scalar.mul(o_t[:, b, 0:512], pc[:],
                                          rinv_g[g][:, b:b + 1])
                        else:
                            nc.vector.tensor_scalar_mul(o_t[:, b, 0:512],
                                                        pc[:],
                                                        rinv_g[g][:, b:b + 1])
                        _n["o3"] += 1
                        o3e = (nc.vector if _n["o3"] <= o["o3_dve"]
                               else nc.gpsimd)
                        o3e.tensor_mul(o_t[:, b, 512:1024], c_fine(j),
                                       o_t[:, b, 0:512])
                        if dma_fine:
                            out_dma(ors[:, j, 512:1536], o_t[:, b, :])
                    if not dma_fine:
                        out_dma(ors[:, 4 * g:4 * g + 4, 512:1536], o_t[:])
                    # o1 echo rides behind this group's output as pipe
                    # filler: DRAM->DRAM, always ready, fills DMA idle.
                    out_dma(ors[:, 4 * g:4 * g + 4, 0:512],
                            crs[:, 4 * g:4 * g + 4, :])

                def o4_block(gs, q2c_bc, dma_fine=False):
                    for g in gs:
                        for b in range(4):
                            j = 4 * g + b
                            _n["o4"] += 1
                            o4e = (nc.vector if _n["o4"] <= o["o4_dve"]
                                   else nc.gpsimd)
                            o4e.tensor_mul(o4_g[g][:, b, :], c_fine(j),
                                           q2c_bc[:])
                            if dma_fine:
                                out_dma(ors[:, j, 1536:2048],
                                        o4_g[g][:, b, :])
                        if not dma_fine:
                            out_dma(ors[:, 4 * g:4 * g + 4, 1536:2048],
                                    o4_g[g][:])

                def q2c_finalize():
                    zinv = constp.tile([128, 1], f32, tag="zinv")
                    nc.vector.reciprocal(zinv[:], zacc[:])
                    q2c_u = constp.tile([1, D], bf16, tag="q2c_u")
                    nc.vector.tensor_copy(q2c_u[:], psq2c[0][:])
                    psbc = ps_q2c.tile([128, D], f32, tag="ps_q2c")
                    nc.tensor.matmul(psbc[:], ones_row_b[:], q2c_u[:],
                                     start=True, stop=True,
                                     skip_group_check=True)
                    q2c_bc = constp.tile([128, D], f32, tag="q2c_bc")
                    nc.scalar.mul(q2c_bc[:], psbc[:], zinv[:])
                    return q2c_bc

                # ---------------- pipelined emission -----------------------
                # mm2 one group behind phase1 so o2/o3 bytes flow early;
                # o4 streams as soon as the beta reduction closes.
                phase1(0)
                phase1(1)
                q2c_partials(0)
                mm2_block(0)
                phase1(2)
                q2c_partials(1)
                mm2_block(1)
                phase1(3)
                q2c_partials(2)
                mm2_block(2)
                q2c_partials(3)
                q2c_bc = q2c_finalize()
                o4_block([0, 1, 2], q2c_bc)
                mm2_block(3, dma_fine=True)
                o4_block([3], q2c_bc, dma_fine=True)

        if timing_mode:
            with tc.tile_pool(name="tickp", bufs=1) as tickp:
                tk = tickp.tile([1, 1], f32, tag="tick")
                nc.vector.memset(tk[:], 1.0)
                nc.sync.dma_start(tick_d[:], tk[:])

    nc.compile()
    return nc


def _get_built():
    global _BUILT
    if _BUILT is None:
        _BUILT = _build()
    return _BUILT


def kernel(c, q, w_c, b_c, w_q, b_q, w_cq, b_cq):
    """Full inputs in, full output out. Data-parallel over batch on 8 cores.

    Biases cancel mathematically (softmax shift invariance), so b_* are
    accepted but unused.
    """
    from concourse import bass_utils

    nc = _get_built()
    c = np.ascontiguousarray(np.asarray(c, dtype=np.float32))
    q = np.ascontiguousarray(np.asarray(q, dtype=np.float32))
    wc = np.ascontiguousarray(np.asarray(w_c, dtype=np.float32))
    wq = np.ascontiguousarray(np.asarray(w_q, dtype=np.float32))
    wcq = np.ascontiguousarray(np.asarray(w_cq, dtype=np.float32))

    in_maps = [
        {"c": c[b], "q": q[b], "wc": wc, "wq": wq, "wcq": wcq}
        for b in range(N_CORES)
    ]
    res = bass_utils.run_bass_kernel_spmd(
        nc, in_maps, core_ids=list(range(N_CORES)))
    return np.stack([res.results[b]["out"] for b in range(N_CORES)])
